# revision 7
# baseline (speedup 1.0000x reference)
"""Trainium2 Bass kernel for BinConv2d:
   y = relu(conv2d(sign(batchnorm_train(x)), W, pad=1) + b)

Sharding: data-parallel over batch, 4 images per core on 8 cores.
BN statistics are computed per-core (bn_stats/bn_aggr) and combined with a
tiny [128,2] AllReduce; sign() only needs a per-channel affine threshold
(sign(gamma*x + (beta*sigma - gamma*mean))), so the variance path never
touches per-element math.

Conv is 9 "taps" of a 64->64 matmul over all pixels. Binarized activations
(exact +-1 in fp16) are stored zero-padded [64ch, 114*114] per image, plus a
row-shifted duplicate on partitions 64..127 so that taps (kh,kw) and
(kh+1,kw) pair into one K=128 matmul. Two 4-row output chunks run
concurrently on the two column halves of the PE array via tile_position.

Pipeline: x-load DMAs overlap bn_stats; dummy matmuls paced off the load
keep the PE HAM clock warm through the stats + AllReduce bubble so the conv
runs at 2.4 GHz from its first matmul.
"""

import sys
from contextlib import ExitStack

import numpy as np

try:
    import concourse.bass as bass  # noqa: F401
except ImportError:  # pragma: no cover
    sys.path.insert(0, "/opt/trn_rl_repo")

import concourse.bacc as bacc
import concourse.tile as tile
from concourse import mybir
from concourse.bass_utils import run_bass_kernel_spmd
from concourse.tile_rust import add_dep_helper

F32 = mybir.dt.float32
WDT = mybir.dt.float16  # dtype for conv weights and binarized activations

N_CORES = 8
N_IMG = 4  # images per core (batch 32 / 8 cores)
C = 64
H = 112
W = 112
HP = H + 2  # 114
WP = W + 2  # 114
IMG = HP * WP  # 12996
EPS = 1e-4

ROWS_PER_CHUNK = 4  # output rows per matmul chunk (N = 4*112 = 448)
NMM = ROWS_PER_CHUNK * W  # 448

N_DUM_CHAIN = 26  # PE keep-warm matmuls bridging the AllReduce bubble
N_DUM_POST = 8  # PE keep-warm matmuls during binarize of image 0


def build_program(n_cores=N_CORES, n_img=N_IMG):
    """Builds the per-core Bass program (same program on every core)."""
    assert n_img % 2 == 0
    nhalf = n_img // 2
    fpart = nhalf * H * W  # free elems per partition of resident x
    n_halves = 2

    nc = bacc.Bacc(
        "TRN2", target_bir_lowering=False, debug=False, num_devices=n_cores
    )

    x = nc.dram_tensor("x", [n_img, C, H, W], F32, kind="ExternalInput")
    gamma = nc.dram_tensor("gamma", [C], F32, kind="ExternalInput")
    beta = nc.dram_tensor("beta", [C], F32, kind="ExternalInput")
    Wt = nc.dram_tensor("W", [C, C, 3, 3], F32, kind="ExternalInput")
    bt = nc.dram_tensor("b", [C], F32, kind="ExternalInput")
    y = nc.dram_tensor("y", [n_img, C, H, W], F32, kind="ExternalOutput")

    with tile.TileContext(nc) as tc, ExitStack() as ctx:
        const = ctx.enter_context(tc.tile_pool(name="const", bufs=1))
        bigp = ctx.enter_context(tc.tile_pool(name="big", bufs=1))
        xbp = ctx.enter_context(tc.tile_pool(name="xb", bufs=2))
        statp = ctx.enter_context(tc.tile_pool(name="stat", bufs=1))
        psump = ctx.enter_context(tc.tile_pool(name="ps", bufs=4, space="PSUM"))
        psdum = ctx.enter_context(tc.tile_pool(name="psd", bufs=1, space="PSUM"))
        outp = ctx.enter_context(tc.tile_pool(name="out", bufs=4))
        dramp = ctx.enter_context(tc.tile_pool(name="dram", bufs=1, space="DRAM"))

        # dummy lhsT for PE keep-warm matmuls
        wdum = const.tile([128, C], F32)
        nc.gpsimd.memset(wdum, 1.0)

        # ---------------- load x + local BN stats (pipelined) -------------
        # resident x: partition p = 64*half + c ; free = n2*12544 + h*112 + w
        xsb = bigp.tile([128, fpart], F32)
        xsb_v = xsb.rearrange("p (n2 h w) -> p n2 h w", n2=nhalf, h=H)

        n_bn = fpart // NMM  # bn_stats chunks of 448
        stats = statp.tile([128, n_bn, 6], F32)
        q_rows = 28  # DMA chunk rows
        n_q = H // q_rows
        bn_per_q = q_rows * W // NMM  # 7
        dum_i = 0

        def dummy_mm(rhs_base):
            nonlocal dum_i
            psD = psdum.tile([C, NMM], F32, tag="psd")
            mm = nc.tensor.matmul(
                psD,
                wdum[:, :],
                xsb[:, rhs_base : rhs_base + NMM],
                start=True,
                stop=True,
                skip_group_check=True,
            )
            dum_i += 1
            return mm

        for n2 in range(nhalf):
            for q in range(n_q):
                for half in range(n_halves):
                    n = half * nhalf + n2
                    base = n2 * (H * W) + q * (q_rows * W)
                    dst = xsb[
                        half * C : half * C + C, base : base + q_rows * W
                    ].rearrange("c (h w) -> c h w", w=W)
                    nc.sync.dma_start(
                        out=dst,
                        in_=x.ap()[n, :, q * q_rows : (q + 1) * q_rows, :],
                    )
                for j in range(bn_per_q):
                    idx = (n2 * n_q + q) * bn_per_q + j
                    base = n2 * (H * W) + q * (q_rows * W) + j * NMM
                    nc.vector.bn_stats(
                        out=stats[:, idx, :], in_=xsb[:, base : base + NMM]
                    )
                # keep-warm matmuls paced by chunk arrival
                base = n2 * (H * W) + q * (q_rows * W)
                dummy_mm(base)
                dummy_mm(base + NMM)

        # ---------------- weights / constants (parallel queues) -----------
        # wstage[c, kh, kw, o] staging (f32) then cast to fp16 on DVE later.
        wstage = const.tile([C, 3, 3, C], F32)
        w_src = Wt.ap().rearrange("o c kh kw -> c kh kw o")
        for kh in range(3):
            for kw in range(3):
                nc.scalar.dma_start(
                    out=wstage[:, kh, kw, :], in_=w_src[:, kh, kw, :]
                )
        b2 = const.tile([128, 1], F32)
        bsrc = bt.ap().rearrange("(c u) -> c u", u=1)
        nc.scalar.dma_start(out=b2[0:C, :], in_=bsrc)
        nc.scalar.dma_start(out=b2[C:128, :], in_=bsrc)
        gamma2 = const.tile([128, 1], F32)
        gsrc = gamma.ap().rearrange("(c u) -> c u", u=1)
        nc.scalar.dma_start(out=gamma2[0:C, :], in_=gsrc)
        nc.scalar.dma_start(out=gamma2[C:128, :], in_=gsrc)
        beta64 = const.tile([C, 1], F32)
        nc.scalar.dma_start(out=beta64, in_=beta.ap().rearrange("(c u) -> c u", u=1))
        eps64 = const.tile([C, 1], F32)
        nc.gpsimd.memset(eps64, EPS)

        # ---------------- aggregate + AllReduce ----------------
        mv = statp.tile([128, 2], F32)
        nc.vector.bn_aggr(out=mv, in_=stats)
        # ar payload: col0 = mean_p, col1 = E[x^2]_p = var_p + mean_p^2
        arin = statp.tile([128, 2], F32)
        nc.vector.tensor_copy(out=arin[:, 0:1], in_=mv[:, 0:1])
        msq = statp.tile([128, 1], F32)
        nc.vector.tensor_mul(out=msq, in0=mv[:, 0:1], in1=mv[:, 0:1])
        nc.vector.tensor_add(out=arin[:, 1:2], in0=mv[:, 1:2], in1=msq)

        cc_in = dramp.tile([128, 2], F32)
        cc_out = dramp.tile([128, 2], F32)
        nc.sync.dma_start(out=cc_in, in_=arin)
        if n_cores > 1:
            nc.gpsimd.collective_compute(
                "AllReduce",
                mybir.AluOpType.add,
                replica_groups=[list(range(n_cores))],
                ins=[cc_in[:].opt()],
                outs=[cc_out[:].opt()],
            )
        else:
            nc.gpsimd.dma_start(out=cc_out, in_=cc_in)
        ar = statp.tile([128, 2], F32)
        ar_dma = nc.sync.dma_start(out=ar, in_=cc_out)

        # fp16 weight views: w2[0:64,t,:] = tap t; w2[64:128,t,:] = tap t+3
        w2 = const.tile([128, 9, C], WDT)
        wsv = wstage.rearrange("c kh kw o -> c (kh kw) o")
        nc.vector.tensor_copy(out=w2[0:C, :, :], in_=wsv)
        nc.vector.tensor_copy(out=w2[C:128, 0:6, :], in_=w2[0:C, 3:9, :])

        # PE keep-warm chain across the AllReduce bubble (no dep on AR)
        for i in range(N_DUM_CHAIN):
            dummy_mm((i % n_bn) * NMM)

        # ---------------- fold stats -> per-channel threshold --------------
        n_groups = n_cores * n_halves
        hi = statp.tile([C, 2], F32)
        nc.scalar.activation(
            out=hi, in_=ar[C:128, :], func=mybir.ActivationFunctionType.Copy
        )
        tot = statp.tile([C, 2], F32)
        nc.vector.tensor_add(out=tot, in0=ar[0:C, :], in1=hi)
        mean64 = statp.tile([C, 1], F32)
        nc.vector.tensor_scalar_mul(mean64, tot[:, 0:1], 1.0 / n_groups)
        e2 = statp.tile([C, 1], F32)
        nc.vector.tensor_scalar_mul(e2, tot[:, 1:2], 1.0 / n_groups)
        var64 = statp.tile([C, 1], F32)
        nc.vector.tensor_mul(out=var64, in0=mean64, in1=mean64)
        nc.vector.tensor_sub(out=var64, in0=e2, in1=var64)
        sigma = statp.tile([C, 1], F32)
        nc.scalar.activation(
            out=sigma,
            in_=var64,
            func=mybir.ActivationFunctionType.Sqrt,
            bias=eps64,
        )
        # d = beta*sigma - gamma*mean ; binarize: xb = sign(gamma*x + d)
        d64 = statp.tile([C, 1], F32)
        nc.vector.tensor_mul(out=d64, in0=beta64, in1=sigma)
        t2 = statp.tile([C, 1], F32)
        nc.vector.tensor_mul(out=t2, in0=gamma2[0:C, :], in1=mean64)
        nc.vector.tensor_sub(out=d64, in0=d64, in1=t2)
        d2 = statp.tile([128, 1], F32)
        nc.vector.tensor_copy(out=d2[0:C, :], in_=d64)
        nc.scalar.activation(
            out=d2[C:128, :], in_=d64, func=mybir.ActivationFunctionType.Copy
        )

        # PE keep-warm during binarize of image 0 (gated on AR completion)
        for i in range(N_DUM_POST):
            mm = dummy_mm((i % n_bn) * NMM)
            add_dep_helper(mm.ins, ar_dma.ins, reason="keep PE warm until AR lands")

        # ---------------- per image: binarize + conv ----------------
        h_split = 56  # binarize row chunks: rows [0,56) then [56,112)
        for n in range(n_img):
            half = n // nhalf
            n2 = n % nhalf
            xbt = xbp.tile([128, IMG], WDT, tag="xb")
            xbv = xbt.rearrange("p (hp wp) -> p hp wp", wp=WP)
            # zero borders of copy A (copy B inherits them)
            nc.gpsimd.memset(xbv[0:C, 0:1, :], 0.0)
            nc.gpsimd.memset(xbv[0:C, HP - 1 : HP, :], 0.0)
            nc.gpsimd.memset(xbv[0:C, 1 : HP - 1, 0:1], 0.0)
            nc.gpsimd.memset(xbv[0:C, 1 : HP - 1, WP - 1 : WP], 0.0)
            # binarize interior in 2 row-chunks: xb = Sign(gamma * x + d)
            for h0c, h1c in ((0, h_split), (h_split, H)):
                nc.scalar.activation(
                    out=xbv[0:C, 1 + h0c : 1 + h1c, 1 : WP - 1],
                    in_=xsb_v[half * C : half * C + C, n2, h0c:h1c, :],
                    func=mybir.ActivationFunctionType.Sign,
                    scale=gamma2[half * C : half * C + C, :],
                    bias=d2[half * C : half * C + C, :],
                )
                # copy B: partitions 64..127 = copy A shifted one padded row.
                # B row b = A row b+1, so chunk (h0c,h1c) (pad rows 1+h0c..h1c
                # written) provides B rows h0c..h1c-1; the last chunk extends
                # through B row 112 (A row 113 is the zeroed border).
                lo = h0c * WP
                hi_ = h1c * WP if h1c < H else IMG - WP
                nc.gpsimd.tensor_copy(
                    out=xbt[C:128, lo:hi_], in_=xbt[0:C, lo + WP : hi_ + WP]
                )

            n_slots = H // (2 * ROWS_PER_CHUNK)  # 14
            for s in range(n_slots):
                h0 = s * 2 * ROWS_PER_CHUNK
                h1 = h0 + ROWS_PER_CHUNK
                P = psump.tile([128, NMM], F32, tag="psum")
                mms = []
                # pairs (kh=0&1) then solos (kh=2); col groups interleaved
                for kw in range(3):
                    for cg, hb in ((0, h0), (64, h1)):
                        mms.append((cg, hb, kw, True))
                for kw in range(3):
                    for cg, hb in ((0, h0), (64, h1)):
                        mms.append((cg, hb, kw, False))
                cg_seen = set()
                cg_last = {cg: max(i for i, m in enumerate(mms) if m[0] == cg)
                           for cg in (0, 64)}
                for i, (cg, hb, kw, is_pair) in enumerate(mms):
                    if is_pair:
                        lhsT = w2[:, kw, :]
                        rhs = xbv[:, hb : hb + ROWS_PER_CHUNK, kw : kw + W]
                    else:
                        lhsT = w2[0:C, 6 + kw, :]
                        rhs = xbv[
                            0:C, hb + 2 : hb + 2 + ROWS_PER_CHUNK, kw : kw + W
                        ]
                    nc.tensor.matmul(
                        P[cg : cg + C, :],
                        lhsT,
                        rhs,
                        start=(cg not in cg_seen),
                        stop=(i == cg_last[cg]),
                        tile_position=(0, cg),
                        skip_group_check=True,
                    )
                    cg_seen.add(cg)
                # epilogue relu(P + b): alternate engines by slot parity
                osb = outp.tile([128, NMM], F32, tag="osb")
                if s % 2 == 0:
                    nc.scalar.activation(
                        out=osb,
                        in_=P,
                        func=mybir.ActivationFunctionType.Relu,
                        bias=b2,
                    )
                else:
                    nc.vector.tensor_scalar(
                        out=osb,
                        in0=P,
                        scalar1=b2,
                        scalar2=0.0,
                        op0=mybir.AluOpType.add,
                        op1=mybir.AluOpType.max,
                    )
                ov = osb.rearrange("p (h w) -> p h w", w=W)
                nc.sync.dma_start(
                    out=y.ap()[n, :, h0 : h0 + ROWS_PER_CHUNK, :],
                    in_=ov[0:C, :, :],
                )
                nc.sync.dma_start(
                    out=y.ap()[n, :, h1 : h1 + ROWS_PER_CHUNK, :],
                    in_=ov[C:128, :, :],
                )

    nc.compile()
    return nc


_CACHE = {}


def _get_program(n_cores=N_CORES, n_img=N_IMG):
    key = (n_cores, n_img)
    if key not in _CACHE:
        _CACHE[key] = build_program(n_cores, n_img)
    return _CACHE[key]


def kernel(x, gamma, beta, W, b, _trace=False):
    x = np.ascontiguousarray(x, dtype=np.float32)
    n_total = x.shape[0]
    assert n_total == N_CORES * N_IMG, x.shape
    nc = _get_program(N_CORES, N_IMG)
    in_maps = []
    for c in range(N_CORES):
        in_maps.append(
            {
                "x": x[c * N_IMG : (c + 1) * N_IMG],
                "gamma": np.ascontiguousarray(gamma, np.float32),
                "beta": np.ascontiguousarray(beta, np.float32),
                "W": np.ascontiguousarray(W, np.float32),
                "b": np.ascontiguousarray(b, np.float32),
            }
        )
    res = run_bass_kernel_spmd(
        nc, in_maps, core_ids=list(range(N_CORES)), trace=_trace
    )
    out = np.concatenate([res.results[c]["y"] for c in range(N_CORES)], axis=0)
    if _trace:
        kernel._last_result = res
    return out


# revision 14
# speedup vs baseline: 1.5224x; 1.5224x over previous
"""Trainium2 Bass kernel for BinConv2d:
   y = relu(conv2d(sign(batchnorm_train(x)), W, pad=1) + b)

Sharding: data-parallel over batch, 4 images per core on 8 cores.
BN statistics are computed per-core (bn_stats/bn_aggr) and combined with a
tiny [128,2] AllReduce; sign() only needs a per-channel affine threshold
(sign(gamma*x + (beta*sigma - gamma*mean))), so the variance path never
touches per-element math.

Conv is 9 "taps" of a 64->64 matmul over all pixels. Binarized activations
(exact +-1 in fp16) are stored zero-padded [64ch, 114*114] per image, plus a
row-shifted duplicate on partitions 64..127 so that taps (kh,kw) and
(kh+1,kw) pair into one K=128 matmul. Two 4-row output chunks run
concurrently on the two column halves of the PE array via tile_position.

Pipeline: x-load DMAs overlap bn_stats; dummy matmuls paced off the load
keep the PE HAM clock warm through the stats + AllReduce bubble so the conv
runs at 2.4 GHz from its first matmul.
"""

import sys
from contextlib import ExitStack

import numpy as np

try:
    import concourse.bass as bass  # noqa: F401
except ImportError:  # pragma: no cover
    sys.path.insert(0, "/opt/trn_rl_repo")

import concourse.bacc as bacc
import concourse.tile as tile
from concourse import mybir
from concourse.bass_utils import run_bass_kernel_spmd
from concourse.masks import make_identity

F32 = mybir.dt.float32
WDT = mybir.dt.float16  # dtype for conv weights and binarized activations

N_CORES = 8
N_IMG = 4  # images per core (batch 32 / 8 cores)
C = 64
H = 112
W = 112
HP = H + 2  # 114
WP = W + 2  # 114
IMG = HP * WP  # 12996
EPS = 1e-4

ROWS_PER_CHUNK = 4  # output rows per matmul chunk (N = 4*112 = 448)
NMM = ROWS_PER_CHUNK * W  # 448

N_DUM_CHAIN = 26  # PE keep-warm matmuls bridging the AllReduce bubble
N_DUM_POST = 8  # PE keep-warm matmuls during binarize of image 0


def build_program(n_cores=N_CORES, n_img=N_IMG):
    """Builds the per-core Bass program (same program on every core)."""
    assert n_img % 2 == 0
    nhalf = n_img // 2
    fpart = nhalf * H * W  # free elems per partition of resident x
    n_halves = 2

    nc = bacc.Bacc(
        "TRN2", target_bir_lowering=False, debug=False, num_devices=n_cores
    )

    x = nc.dram_tensor("x", [n_img, C, H, W], F32, kind="ExternalInput")
    gamma = nc.dram_tensor("gamma", [C], F32, kind="ExternalInput")
    beta = nc.dram_tensor("beta", [C], F32, kind="ExternalInput")
    Wt = nc.dram_tensor("W", [C, C, 3, 3], F32, kind="ExternalInput")
    bt = nc.dram_tensor("b", [C], F32, kind="ExternalInput")
    y = nc.dram_tensor("y", [n_img, C, H, W], F32, kind="ExternalOutput")

    with tile.TileContext(nc) as tc, ExitStack() as ctx:
        const = ctx.enter_context(tc.tile_pool(name="const", bufs=1))
        bigp = ctx.enter_context(tc.tile_pool(name="big", bufs=1))
        xbp = ctx.enter_context(tc.tile_pool(name="xb", bufs=2))
        statp = ctx.enter_context(tc.tile_pool(name="stat", bufs=1))
        psump = ctx.enter_context(tc.tile_pool(name="ps", bufs=4, space="PSUM"))
        psdum = ctx.enter_context(tc.tile_pool(name="psd", bufs=1, space="PSUM"))
        pstr = ctx.enter_context(tc.tile_pool(name="pst", bufs=2, space="PSUM"))
        outp = ctx.enter_context(tc.tile_pool(name="out", bufs=4))
        dramp = ctx.enter_context(tc.tile_pool(name="dram", bufs=1, space="DRAM"))

        # dummy lhsT for PE keep-warm matmuls; wdum2 re-written (trivially)
        # from the AllReduce result so post-AR warm-up matmuls wait for it
        wdum = const.tile([128, C], F32)
        nc.gpsimd.memset(wdum, 1.0)
        wdum2 = const.tile([128, C], F32)
        nc.gpsimd.memset(wdum2, 1.0)
        identity64 = const.tile([C, C], F32)
        make_identity(nc, identity64)
        eps64 = const.tile([C, 1], F32)
        nc.gpsimd.memset(eps64, EPS)

        # warm-up AllReduce: aligns the 8 cores and wakes the collective
        # path so the real (blocking) AllReduce later starts faster
        if n_cores > 1:
            cc_w_in = dramp.tile([C, 1], F32)
            cc_w_out = dramp.tile([C, 1], F32)
            nc.gpsimd.dma_start(out=cc_w_in, in_=eps64)
            nc.gpsimd.collective_compute(
                "AllReduce",
                mybir.AluOpType.add,
                replica_groups=[list(range(n_cores))],
                ins=[cc_w_in[:].opt()],
                outs=[cc_w_out[:].opt()],
            )

        # ---------------- load x + local BN stats (pipelined) -------------
        # resident x: partition p = 64*half + c ; free = n2*12544 + h*112 + w
        xsb = bigp.tile([128, fpart], F32)
        xsb_v = xsb.rearrange("p (n2 h w) -> p n2 h w", n2=nhalf, h=H)

        n_bn = fpart // NMM  # bn_stats chunks of 448
        stats = statp.tile([128, n_bn, 6], F32)
        q_rows = 28  # DMA chunk rows
        n_q = H // q_rows
        bn_per_q = q_rows * W // NMM  # 7
        dum_i = 0

        def dummy_mm(rhs_base, lhsT=None):
            nonlocal dum_i
            psD = psdum.tile([C, NMM], F32, tag="psd")
            mm = nc.tensor.matmul(
                psD,
                wdum[:, :] if lhsT is None else lhsT,
                xsb[:, rhs_base : rhs_base + NMM],
                start=True,
                stop=True,
                skip_group_check=True,
            )
            dum_i += 1
            return mm

        for n2 in range(nhalf):
            for q in range(n_q):
                for half in range(n_halves):
                    n = half * nhalf + n2
                    base = n2 * (H * W) + q * (q_rows * W)
                    dst = xsb[
                        half * C : half * C + C, base : base + q_rows * W
                    ].rearrange("c (h w) -> c h w", w=W)
                    nc.sync.dma_start(
                        out=dst,
                        in_=x.ap()[n, :, q * q_rows : (q + 1) * q_rows, :],
                    )
                for j in range(bn_per_q):
                    idx = (n2 * n_q + q) * bn_per_q + j
                    base = n2 * (H * W) + q * (q_rows * W) + j * NMM
                    nc.vector.bn_stats(
                        out=stats[:, idx, :], in_=xsb[:, base : base + NMM]
                    )
                # keep-warm matmuls paced by chunk arrival
                base = n2 * (H * W) + q * (q_rows * W)
                dummy_mm(base)
                dummy_mm(base + NMM)

        # ---------------- weights / constants (parallel queues) -----------
        # W loads contiguously as [o, (c kh kw)]; per-tap 64x64 transposes
        # on the PE produce lhsT[c, o] which is cast to fp16.
        wsb = const.tile([C, C, 9], F32)
        nc.scalar.dma_start(
            out=wsb, in_=Wt.ap().rearrange("o c kh kw -> o c (kh kw)")
        )
        b2 = const.tile([128, 1], F32)
        bsrc = bt.ap().rearrange("(c u) -> c u", u=1)
        nc.scalar.dma_start(out=b2[0:C, :], in_=bsrc)
        nc.scalar.dma_start(out=b2[C:128, :], in_=bsrc)
        gamma2 = const.tile([128, 1], F32)
        gsrc = gamma.ap().rearrange("(c u) -> c u", u=1)
        nc.scalar.dma_start(out=gamma2[0:C, :], in_=gsrc)
        nc.scalar.dma_start(out=gamma2[C:128, :], in_=gsrc)
        beta64 = const.tile([C, 1], F32)
        nc.scalar.dma_start(out=beta64, in_=beta.ap().rearrange("(c u) -> c u", u=1))

        # fp16 weight views: w2[0:64,t,:] = tap t; w2[64:128,t,:] = tap t+3
        w2 = const.tile([128, 9, C], WDT)
        for t in range(9):
            psT = pstr.tile([C, C], F32, tag="pst")
            nc.tensor.transpose(psT, wsb[:, :, t], identity64)
            nc.scalar.activation(
                out=w2[0:C, t, :], in_=psT,
                func=mybir.ActivationFunctionType.Copy,
            )
            if t >= 3:
                nc.scalar.activation(
                    out=w2[C:128, t - 3, :], in_=psT,
                    func=mybir.ActivationFunctionType.Copy,
                )

        # ---------------- aggregate + AllReduce ----------------
        mv = statp.tile([128, 2], F32)
        nc.vector.bn_aggr(out=mv, in_=stats)
        # ar payload: col0 = mean_p, col1 = E[x^2]_p = var_p + mean_p^2
        arin = statp.tile([128, 2], F32)
        nc.vector.tensor_copy(out=arin[:, 0:1], in_=mv[:, 0:1])
        msq = statp.tile([128, 1], F32)
        nc.vector.tensor_mul(out=msq, in0=mv[:, 0:1], in1=mv[:, 0:1])
        nc.vector.tensor_add(out=arin[:, 1:2], in0=mv[:, 1:2], in1=msq)

        cc_in = dramp.tile([128, 2], F32)
        cc_out = dramp.tile([128, 2], F32)
        nc.sync.dma_start(out=cc_in, in_=arin)
        if n_cores > 1:
            nc.gpsimd.collective_compute(
                "AllReduce",
                mybir.AluOpType.add,
                replica_groups=[list(range(n_cores))],
                ins=[cc_in[:].opt()],
                outs=[cc_out[:].opt()],
            )
        else:
            nc.gpsimd.dma_start(out=cc_out, in_=cc_in)
        ar = statp.tile([128, 2], F32)
        nc.sync.dma_start(out=ar, in_=cc_out)
        # post-AR warm-up matmuls gate on wdum2, which depends on ar
        nc.vector.tensor_scalar_mul(wdum2[:, 0:2], ar, 0.0)

        # PE keep-warm chain across the AllReduce bubble (no dep on AR)
        for i in range(N_DUM_CHAIN):
            dummy_mm((i % n_bn) * NMM)

        # ---------------- fold stats -> per-channel threshold --------------
        n_groups = n_cores * n_halves
        hi = statp.tile([C, 2], F32)
        nc.scalar.activation(
            out=hi, in_=ar[C:128, :], func=mybir.ActivationFunctionType.Copy
        )
        tot = statp.tile([C, 2], F32)
        nc.vector.tensor_add(out=tot, in0=ar[0:C, :], in1=hi)
        mean64 = statp.tile([C, 1], F32)
        nc.vector.tensor_scalar_mul(mean64, tot[:, 0:1], 1.0 / n_groups)
        e2 = statp.tile([C, 1], F32)
        nc.vector.tensor_scalar_mul(e2, tot[:, 1:2], 1.0 / n_groups)
        var64 = statp.tile([C, 1], F32)
        nc.vector.tensor_mul(out=var64, in0=mean64, in1=mean64)
        nc.vector.tensor_sub(out=var64, in0=e2, in1=var64)
        sigma = statp.tile([C, 1], F32)
        nc.scalar.activation(
            out=sigma,
            in_=var64,
            func=mybir.ActivationFunctionType.Sqrt,
            bias=eps64,
        )
        # d = beta*sigma - gamma*mean ; binarize: xb = sign(gamma*x + d)
        d64 = statp.tile([C, 1], F32)
        nc.vector.tensor_mul(out=d64, in0=beta64, in1=sigma)
        t2 = statp.tile([C, 1], F32)
        nc.vector.tensor_mul(out=t2, in0=gamma2[0:C, :], in1=mean64)
        nc.vector.tensor_sub(out=d64, in0=d64, in1=t2)
        d2 = statp.tile([128, 1], F32)
        nc.vector.tensor_copy(out=d2[0:C, :], in_=d64)
        nc.scalar.activation(
            out=d2[C:128, :], in_=d64, func=mybir.ActivationFunctionType.Copy
        )

        # PE keep-warm during binarize of image 0 (gated on AR via wdum2)
        for i in range(N_DUM_POST):
            dummy_mm((i % n_bn) * NMM, lhsT=wdum2[:, :])

        # ---------------- per image: binarize + conv ----------------
        h_split = 56  # binarize row chunks: rows [0,56) then [56,112)
        for n in range(n_img):
            half = n // nhalf
            n2 = n % nhalf
            xbt = xbp.tile([128, IMG], WDT, tag="xb")
            xbv = xbt.rearrange("p (hp wp) -> p hp wp", wp=WP)
            # zero borders of copy A (copy B inherits them)
            nc.gpsimd.memset(xbv[0:C, 0:1, :], 0.0)
            nc.gpsimd.memset(xbv[0:C, HP - 1 : HP, :], 0.0)
            nc.gpsimd.memset(xbv[0:C, 1 : HP - 1, 0:1], 0.0)
            nc.gpsimd.memset(xbv[0:C, 1 : HP - 1, WP - 1 : WP], 0.0)
            # binarize interior in 2 row-chunks: xb = Sign(gamma * x + d)
            for h0c, h1c in ((0, h_split), (h_split, H)):
                nc.scalar.activation(
                    out=xbv[0:C, 1 + h0c : 1 + h1c, 1 : WP - 1],
                    in_=xsb_v[half * C : half * C + C, n2, h0c:h1c, :],
                    func=mybir.ActivationFunctionType.Sign,
                    scale=gamma2[half * C : half * C + C, :],
                    bias=d2[half * C : half * C + C, :],
                )
                # copy B: partitions 64..127 = copy A shifted one padded row.
                # B row b = A row b+1, so chunk (h0c,h1c) (pad rows 1+h0c..h1c
                # written) provides B rows h0c..h1c-1; the last chunk extends
                # through B row 112 (A row 113 is the zeroed border).
                lo = h0c * WP
                hi_ = h1c * WP if h1c < H else IMG - WP
                nc.vector.tensor_copy(
                    out=xbt[C:128, lo:hi_], in_=xbt[0:C, lo + WP : hi_ + WP]
                )

            n_slots = H // (2 * ROWS_PER_CHUNK)  # 14
            for s in range(n_slots):
                h0 = s * 2 * ROWS_PER_CHUNK
                h1 = h0 + ROWS_PER_CHUNK
                P = psump.tile([128, NMM], F32, tag="psum")
                mms = []
                # pairs (kh=0&1) then solos (kh=2); col groups interleaved
                for kw in range(3):
                    for cg, hb in ((0, h0), (64, h1)):
                        mms.append((cg, hb, kw, True))
                for kw in range(3):
                    for cg, hb in ((0, h0), (64, h1)):
                        mms.append((cg, hb, kw, False))
                cg_seen = set()
                cg_last = {cg: max(i for i, m in enumerate(mms) if m[0] == cg)
                           for cg in (0, 64)}
                for i, (cg, hb, kw, is_pair) in enumerate(mms):
                    if is_pair:
                        lhsT = w2[:, kw, :]
                        rhs = xbv[:, hb : hb + ROWS_PER_CHUNK, kw : kw + W]
                    else:
                        lhsT = w2[0:C, 6 + kw, :]
                        rhs = xbv[
                            0:C, hb + 2 : hb + 2 + ROWS_PER_CHUNK, kw : kw + W
                        ]
                    nc.tensor.matmul(
                        P[cg : cg + C, :],
                        lhsT,
                        rhs,
                        start=(cg not in cg_seen),
                        stop=(i == cg_last[cg]),
                        tile_position=(0, cg),
                        skip_group_check=True,
                    )
                    cg_seen.add(cg)
                # epilogue relu(P + b): alternate engines by slot parity
                osb = outp.tile([128, NMM], F32, tag="osb")
                if s % 2 == 0:
                    nc.scalar.activation(
                        out=osb,
                        in_=P,
                        func=mybir.ActivationFunctionType.Relu,
                        bias=b2,
                    )
                else:
                    nc.vector.tensor_scalar(
                        out=osb,
                        in0=P,
                        scalar1=b2,
                        scalar2=0.0,
                        op0=mybir.AluOpType.add,
                        op1=mybir.AluOpType.max,
                    )
                ov = osb.rearrange("p (h w) -> p h w", w=W)
                nc.sync.dma_start(
                    out=y.ap()[n, :, h0 : h0 + ROWS_PER_CHUNK, :],
                    in_=ov[0:C, :, :],
                )
                nc.sync.dma_start(
                    out=y.ap()[n, :, h1 : h1 + ROWS_PER_CHUNK, :],
                    in_=ov[C:128, :, :],
                )

    nc.compile()
    return nc


_CACHE = {}


def _get_program(n_cores=N_CORES, n_img=N_IMG):
    key = (n_cores, n_img)
    if key not in _CACHE:
        _CACHE[key] = build_program(n_cores, n_img)
    return _CACHE[key]


def kernel(x, gamma, beta, W, b, _trace=False):
    x = np.ascontiguousarray(x, dtype=np.float32)
    n_total = x.shape[0]
    assert n_total == N_CORES * N_IMG, x.shape
    nc = _get_program(N_CORES, N_IMG)
    in_maps = []
    for c in range(N_CORES):
        in_maps.append(
            {
                "x": x[c * N_IMG : (c + 1) * N_IMG],
                "gamma": np.ascontiguousarray(gamma, np.float32),
                "beta": np.ascontiguousarray(beta, np.float32),
                "W": np.ascontiguousarray(W, np.float32),
                "b": np.ascontiguousarray(b, np.float32),
            }
        )
    res = run_bass_kernel_spmd(
        nc, in_maps, core_ids=list(range(N_CORES)), trace=_trace
    )
    out = np.concatenate([res.results[c]["y"] for c in range(N_CORES)], axis=0)
    if _trace:
        kernel._last_result = res
    return out
